# revision 7
# baseline (speedup 1.0000x reference)
"""Fake-attention kernel for trn2: 8 cores, one batch element per core.

Per core (batch b): out = softmax(k @ q^T) @ v, with k/q/v = x @ W.T + b.
All heavy matmuls run as fp32r (full-rate on the PE at free-dim >= 256).

Layout strategy (per core):
  xT [f,n]  <- PE-transpose of x chunks
  kT,qT [d,n] = W @ xT   (lhsT = W^T, pre-transposed on host)
  v [m,d]     = xT-chunks as lhsT, rhs = Wv^T  (natural layout)
  scoresT chunk [m=128, n=1024] = qT-slice as lhsT, kT as rhs
  pT = exp(scoresT)  (scalar engine, fp32r out)
  outT [d, n] += v-chunk as lhsT, pT as rhs    (accumulate over m in PSUM)
  denom[n] = sum_m pT via two parallel partial-sum chains (DVE 18 chunks,
             GPSIMD 14), finalized by PE transpose + free-axis reduce.
  out natural = PE-transpose(outT) * (1/denom) + bv

Scheduling: emit order software-pipelines the PE (scores of chunk mc+1
ahead of PV of mc), the per-1024-group setup interleaves with section 0,
and each section's finalize is deferred into the next section's stream.
"""
import numpy as np

B = 8
N = 4096
D = 128
NC = 32          # chunks of 128 along n/m
NSEC = 4         # sections of 1024 along n
SEC = 1024

_cache = {}


def _build():
    import concourse.bass as bass  # noqa
    import concourse.mybir as mybir
    import concourse.tile as tile
    from concourse import bacc

    F32 = mybir.dt.float32
    F32R = mybir.dt.float32r
    Exp = mybir.ActivationFunctionType.Exp
    AX = mybir.AxisListType.X
    ADD = mybir.AluOpType.add
    MUL = mybir.AluOpType.mult

    nc = bacc.Bacc()
    x = nc.declare_dram_parameter("x", [N, D], F32, isOutput=False)
    wp = nc.declare_dram_parameter("wp", [128, 643], F32, isOutput=False)
    y = nc.declare_dram_parameter("y", [N, D], F32, isOutput=True)

    x_dram = x.rearrange("(c p) f -> p c f", p=128)
    y_dram = y.rearrange("(c p) d -> p c d", p=128)

    with tile.TileContext(nc) as tc:
        with (
            tc.tile_pool(name="big", bufs=1) as big,
            tc.tile_pool(name="ptp", bufs=3) as ptp,
            tc.tile_pool(name="wrk", bufs=2) as wrk,
            tc.tile_pool(name="ps", bufs=2, space="PSUM") as psum,
            tc.tile_pool(name="ps1", bufs=2, space="PSUM") as psum1,
        ):
            wp_sb = big.tile([128, 643], F32, tag="wp")
            nc.sync.dma_start(wp_sb[:], wp[:])
            ident = wp_sb[:, 515:643]
            bv_bc = wp_sb[:, 385:513]
            bk = wp_sb[:, 513:514]
            bq = wp_sb[:, 514:515]

            w_r = big.tile([128, 385], F32R, tag="w_r")
            nc.vector.tensor_copy(w_r[:], wp_sb[:, 0:385])
            wkT = w_r[:, 0:128]
            wqT = w_r[:, 128:256]
            wvT = w_r[:, 256:384]

            kT = [None] * 4
            qT = [None] * 4
            v_g = [None] * 4

            def emit_setup(g):
                xn = big.tile([128, 8, 128], F32, tag=f"x_nat{g}")
                nc.sync.dma_start(xn[:], x_dram[:, g * 8:(g + 1) * 8, :])
                xg = big.tile([128, 8, 128], F32R, tag=f"xT{g}")
                tp = psum.tile([128, 1024], F32, tag="sc")
                for j in range(8):
                    nc.tensor.transpose(
                        tp[:, j * 128:(j + 1) * 128], xn[:, j, :], ident
                    )
                nc.vector.tensor_copy(xg[:], tp[:])
                xgf = xg.rearrange("p c f -> p (c f)")

                kg = big.tile([128, 1024], F32R, tag=f"kT{g}")
                psk = psum.tile([128, 1024], F32, tag="sc")
                nc.tensor.matmul(psk[:, 0:512], wkT, xgf[:, 0:512],
                                 start=True, stop=True)
                nc.tensor.matmul(psk[:, 512:1024], wkT, xgf[:, 512:1024],
                                 start=True, stop=True)
                nc.vector.tensor_scalar_add(kg[:], psk[:], bk)

                qg = big.tile([128, 1024], F32R, tag=f"qT{g}")
                psq = psum.tile([128, 1024], F32, tag="sc")
                nc.tensor.matmul(psq[:, 0:512], wqT, xgf[:, 0:512],
                                 start=True, stop=True)
                nc.tensor.matmul(psq[:, 512:1024], wqT, xgf[:, 512:1024],
                                 start=True, stop=True)
                nc.vector.tensor_scalar_add(qg[:], psq[:], bq)

                vg = big.tile([128, 8, 128], F32R, tag=f"v{g}")
                psv = psum.tile([128, 1024], F32, tag="sc")
                for j in range(8):
                    nc.tensor.matmul(
                        psv[:, j * 128:(j + 1) * 128], xg[:, j, :], wvT,
                        start=True, stop=True,
                    )
                nc.vector.tensor_copy(vg[:], psv[:])
                kT[g] = kg
                qT[g] = qg
                v_g[g] = vg

            # denominator chain assignment: DVE gets 18 chunks, GPSIMD 14
            def chain_of(mc):
                return "gp" if (mc % 2 == 1 and mc < 28) else "dve"

            dve_chunks = [mc for mc in range(NC) if chain_of(mc) == "dve"]
            gp_chunks = [mc for mc in range(NC) if chain_of(mc) == "gp"]

            def q_slice(mc):
                return qT[mc // 8][:, (mc % 8) * 128:(mc % 8 + 1) * 128]

            def v_chunk(mc):
                return v_g[mc // 8][:, mc % 8, :]

            emit_setup(0)

            pending_fin = [None]

            def flush_fin():
                if pending_fin[0] is not None:
                    pending_fin[0]()
                    pending_fin[0] = None

            for sec in range(NSEC):
                ps_pv = psum1.tile([128, 1024], F32, tag="pv")
                d_even = wrk.tile([128, 1024], F32, tag="de")
                d_odd = wrk.tile([128, 1024], F32, tag="do")

                def emit_scores(mc, sec=sec):
                    ps_s = psum.tile([128, 1024], F32, tag="sc")
                    q_sl = q_slice(mc)
                    kg = kT[sec]
                    nc.tensor.matmul(ps_s[:, 0:512], q_sl, kg[:, 0:512],
                                     start=True, stop=True)
                    nc.tensor.matmul(ps_s[:, 512:1024], q_sl, kg[:, 512:1024],
                                     start=True, stop=True)
                    return ps_s

                def emit_exp(ps_s):
                    pT = ptp.tile([128, 1024], F32R, tag="pt")
                    nc.scalar.activation(pT[:], ps_s[:], Exp)
                    return pT

                def emit_pv(mc, pT, ps_pv=ps_pv):
                    nc.tensor.matmul(
                        ps_pv[:, 0:512], v_chunk(mc), pT[:, 0:512],
                        start=(mc == 0), stop=(mc == NC - 1),
                        skip_group_check=True,
                    )
                    nc.tensor.matmul(
                        ps_pv[:, 512:1024], v_chunk(mc), pT[:, 512:1024],
                        start=(mc == 0), stop=(mc == NC - 1),
                        skip_group_check=True,
                    )

                def emit_chain(mc, pT, d_even=d_even, d_odd=d_odd):
                    pTf = pT.bitcast(F32)
                    if chain_of(mc) == "dve":
                        if mc == dve_chunks[0]:
                            nc.vector.tensor_copy(d_even[:], pTf[:])
                        else:
                            nc.vector.tensor_tensor(
                                d_even[:], d_even[:], pTf[:], ADD
                            )
                    else:
                        if mc == gp_chunks[0]:
                            nc.gpsimd.tensor_copy(d_odd[:], pTf[:])
                        else:
                            nc.gpsimd.tensor_tensor(d_odd[:], d_odd[:], pTf[:], ADD)

                ps_prev = emit_scores(0)
                pT_prev = emit_exp(ps_prev)
                for mc in range(1, NC):
                    # interleave remaining setup groups into section 0
                    if sec == 0 and mc % 8 == 1 and mc // 8 + 1 < 4:
                        emit_setup(mc // 8 + 1)
                    ps_s = emit_scores(mc)
                    emit_pv(mc - 1, pT_prev)
                    emit_chain(mc - 1, pT_prev)
                    if mc == 4:
                        flush_fin()
                    pT_prev = emit_exp(ps_s)
                emit_pv(NC - 1, pT_prev)
                emit_chain(NC - 1, pT_prev)

                def make_fin(sec=sec, ps_pv=ps_pv, d_even=d_even, d_odd=d_odd):
                    def fin():
                        nc.vector.tensor_tensor(d_even[:], d_even[:], d_odd[:], ADD)
                        o_copy = wrk.tile([128, 1024], F32, tag="oc")
                        nc.vector.tensor_copy(o_copy[:], ps_pv[:])

                        tpd = psum1.tile([128, 1024], F32, tag="pv")
                        for nb in range(8):
                            sl = slice(nb * 128, (nb + 1) * 128)
                            nc.tensor.transpose(tpd[:, sl], d_even[:, sl], ident)
                        denom = wrk.tile([128, 8], F32, tag="dn")
                        nc.vector.reduce_sum(
                            denom[:], tpd.rearrange("p (b l) -> p b l", b=8),
                            axis=AX,
                        )
                        recip = wrk.tile([128, 8], F32, tag="rc")
                        nc.vector.reciprocal(recip[:], denom[:])

                        tpo = psum1.tile([128, 1024], F32, tag="pv")
                        for nb in range(8):
                            sl = slice(nb * 128, (nb + 1) * 128)
                            nc.tensor.transpose(tpo[:, sl], o_copy[:, sl], ident)

                        out_g = big.tile([128, 8, 128], F32, tag=f"out{sec}")
                        tpo_v = tpo.rearrange("p (b l) -> p b l", b=8)
                        recip_bc = recip[:, :, None].to_broadcast((128, 8, 128))
                        bv_bcx = bv_bc[:, None, :].to_broadcast((128, 8, 128))
                        nc.vector.tensor_tensor(out_g[:], tpo_v, recip_bc, MUL)
                        nc.vector.tensor_tensor(out_g[:], out_g[:], bv_bcx, ADD)
                        nc.sync.dma_start(
                            y_dram[:, sec * 8:(sec + 1) * 8, :], out_g[:]
                        )
                    return fin

                pending_fin[0] = make_fin()

            flush_fin()

    nc.finalize()
    return nc


def _get_nc():
    if "nc" not in _cache:
        _cache["nc"] = _build()
    return _cache["nc"]


def make_wp(Wk, Wq, Wv, bk, bq, bv):
    wp = np.zeros((128, 643), np.float32)
    wp[:, 0:128] = Wk.T
    wp[:, 128:256] = Wq.T
    wp[:, 256:384] = Wv.T
    wp[:, 384] = 1.0
    wp[:, 385:513] = np.broadcast_to(bv[None, :], (128, 128))
    wp[:, 513] = bk
    wp[:, 514] = bq
    wp[:, 515:643] = np.eye(128, dtype=np.float32)
    return wp


def kernel(x, Wk, bk, Wq, bq, Wv, bv, **_ignored):
    from concourse.bass_utils import run_bass_kernel_spmd

    x = np.asarray(x, dtype=np.float32)
    wp = make_wp(
        np.asarray(Wk, np.float32), np.asarray(Wq, np.float32),
        np.asarray(Wv, np.float32), np.asarray(bk, np.float32),
        np.asarray(bq, np.float32), np.asarray(bv, np.float32),
    )

    nc = _get_nc()
    in_maps = [
        {"x": np.ascontiguousarray(x[b]), "wp": wp} for b in range(B)
    ]
    res = run_bass_kernel_spmd(nc, in_maps, core_ids=list(range(B)))
    out = np.stack([res.results[b]["y"] for b in range(B)], axis=0)
    return out
